# revision 21
# baseline (speedup 1.0000x reference)
"""EEGGCNet Trainium2 kernel: 8-core batch-parallel Bass/Tile implementation.

Pipeline (per core, local batch of 8):
  conv1 (temporal, 80-tap) -> BN1 -> Chebyshev graph conv + node-mean ->
  BN2 -> ELU -> pool4 -> separable conv (128ch, 16-tap) -> BN3 -> ELU ->
  pool8 -> FC.

Algebraic restructuring (all exact):
  * The Chebyshev recursion + node-mean collapses to a single (16 x 64)
    projection W applied over the node axis (host-precomputed from L and
    cheb_w; polynomials of L commute with L).
  * BN affine constants (conv1_b, bn1_b, cheb_b, sep_b) cancel inside the
    mean-subtractions; only bn variances and means of the *raw* conv
    outputs are needed.
  * BatchNorm batch statistics are reduced across cores with two tiny
    AllReduces (sums + sums-of-squares).
  * avgpool divisors are folded into downstream weights.

Layouts:
  * conv1-for-stats runs time-major via PE transposes + banded-Toeplitz
    stationary matrices (host-built from conv1_w).
  * The projected conv (G path) runs channel-major using an 8x shifted
    replication of the projected signal xw; contraction dim = (d, tau).
  * sep conv is a plain 16-tap accumulation matmul over channels.
"""

import sys

sys.path.insert(0, "/opt/trn_rl_repo")

import numpy as np

import concourse.bass as bass
import concourse.mybir as mybir
from concourse import tile
from concourse.vector_clock import ScopedClock
from concourse.bass_utils import run_bass_kernel_spmd

F1, D, K, CH, T, KL, NCLS = 8, 16, 5, 64, 640, 80, 4
B, NCORES, BL = 64, 8, 8
EPS = 1e-5
T2, T3 = 160, 20
BNT = float(B * CH * T)
BT = float(B * T)
BT2 = float(B * T2)
F32 = mybir.dt.float32
FR = mybir.dt.float32r
AL = mybir.AluOpType
AF = mybir.ActivationFunctionType
_MM_DT = "f32r"  # "f32" | "f32r"
_PHASES = 99  # emission truncation for bisection

# Tile starts for the time-major stats conv: tile 0 at t=0 holds output
# blocks t0 in {0,16,32,48,64}; tiles i=1..12 at s=41+48(i-1) hold blocks
# s+{39,55,71}.  8 distinct Toeplitz phases.
PHI = [0, 16, 32, 48, 64, 39, 55, 71]
TILE_S = [0] + [41 + 48 * (i - 1) for i in range(1, 13)]


def _blk_tile_phi(t0):
    if t0 <= 64:
        return 0, PHI.index(t0)
    i = (t0 - 80) // 48 + 1
    return i, PHI.index(t0 - TILE_S[i])


def _patch_tile_tail():
    """This walrus build allows at most ~2 sync waits on TPB_CTRL-class
    instructions (Drain) and rejects sem-eq-imm waits on them.  Split the
    TileContext tail-drain waits one-per-nop and use the sem-only
    barrier."""

    def _drain_and_barrier(self, tick_clock, wait_clock):
        nc = self.nc
        probe = nc.sync.nop(nofuse=True)
        wait_clock.add_sem_waits(
            probe.ins, ScopedClock({None: tick_clock.global_clock})
        )
        si = probe.ins.sync_info
        waits = list(si.on_wait or []) if si is not None else []
        if si is not None:
            si.on_wait = waits[:1]
        for w in waits[1:]:
            n2 = nc.sync.nop(nofuse=True)
            if n2.ins.sync_info is None:
                n2.ins.sync_info = mybir.SyncInfo(on_wait=[w], on_update=[])
            else:
                n2.ins.sync_info.on_wait = [w]
        nc.sync.drain()
        nc.all_engine_barrier(sem_only=True)
        popped = nc._tile_sem_poison_stack.pop()
        assert popped is self._sem_poison
        sems = list(self.sems.allocated().values())
        for i in range(0, len(sems), 8):
            nc.clear_and_free_semaphores(sems[i : i + 8])
        nc.all_engine_barrier(sem_only=True)

    tile.TileContext._drain_and_barrier = _drain_and_barrier


def _split_waits(nc, limit=1):
    """This walrus build supports very few sync-wait slots per instruction.
    Hoist excess waits onto same-engine nops placed immediately before the
    instruction (engine queues are FIFO, so semantics are preserved)."""
    ctr = [0]
    for bb in nc.main_func.blocks:
        new = []
        for ins in bb.instructions:
            si = ins.sync_info
            waits = list(si.on_wait) if (si is not None and si.on_wait) else []
            if len(waits) > limit:
                for w in waits[:-limit]:
                    ctr[0] += 1
                    nop = mybir.InstNoOp(name=f"wsplit_{ctr[0]}", ins=[], outs=[])
                    nop.engine = ins.engine
                    nop.sync_info = mybir.SyncInfo(on_wait=[w], on_update=[])
                    nc.register_instruction(nop, overwrite=True)
                    new.append(nop)
                si.on_wait = waits[-limit:]
            new.append(ins)
        bb.instructions[:] = new


def _host_tensors(L, conv1_w, cheb_w, sep_w, fc_w):
    w1 = np.asarray(conv1_w, np.float64)[:, 0, 0, :]  # (8, 80)
    Lm = np.asarray(L, np.float64)

    # Chebyshev node-mean row vectors: V[k] = (1/N) 1^T P_k(L)
    V = np.zeros((K, CH))
    V[0] = 1.0 / CH
    V[1] = V[0] @ Lm
    for k in range(2, K):
        V[k] = 2.0 * (V[k - 1] @ Lm) - V[k - 2]
    W = np.asarray(cheb_w, np.float64)[:, 0, :].T @ V  # (16, 64)

    # Block-diag projection stationary (two batches per matmul).
    wd = np.zeros((128, 32))
    for j in range(2):
        wd[j * 64 : (j + 1) * 64, j * 16 : (j + 1) * 16] = W.T

    # Toeplitz stationaries for the time-major stats conv.
    toep = np.zeros((128, 8 * 128))
    for p, phi in enumerate(PHI):
        for f in range(F1):
            for j in range(16):
                for r in range(128):
                    tap = r - j - phi + 39
                    if 0 <= tap < KL:
                        toep[r, p * 128 + f * 16 + j] = w1[f, tap]

    # Channel-major G-conv stationaries: k = (d, tau8), m = (f, d).
    sg = np.zeros((128, 10 * 128))
    for g in range(10):
        for d in range(D):
            for tau in range(8):
                for f in range(F1):
                    sg[d * 8 + tau, g * 128 + f * 16 + d] = w1[f, 8 * g + tau]

    # sep conv stationaries per tap, pool4 divisor folded in.
    w2t = np.zeros((128, 16 * 128))
    sw = np.asarray(sep_w, np.float64)[:, :, 0, :]  # (oc, ic, tap)
    for tap in range(16):
        w2t[:, tap * 128 : (tap + 1) * 128] = sw[:, :, tap].T / 4.0

    # FC stationaries per pooled-time index, pool8 divisor folded in.
    fcw = np.zeros((128, 20 * 4))
    fw = np.asarray(fc_w, np.float64)  # (4, 2560), flat = c*20 + t''
    for t2 in range(20):
        for j in range(NCLS):
            fcw[:, t2 * 4 + j] = fw[j, t2::20] / 8.0

    # Banded ones for the windowed sums u[tau] (sum_y path).
    bones = np.zeros((128, 5 * 80))
    for q in range(5):
        for r in range(128):
            s = q * 128 + r
            for tau in range(KL):
                if tau - 39 <= s <= 600 + tau:
                    bones[r, q * 80 + tau] = 1.0

    of = np.zeros((128, 8))
    for f in range(F1):
        of[f * 16 : (f + 1) * 16, f] = 1.0
    bc8 = np.zeros((8, 128))
    for f in range(F1):
        bc8[f, f * 16 : (f + 1) * 16] = 1.0

    t = lambda a: np.ascontiguousarray(a, np.float32)
    return {
        "toepT": t(toep),
        "sg": t(sg),
        "wd": t(wd),
        "w2t": t(w2t),
        "fcw": t(fcw),
        "bones": t(bones),
        "w1t": t(w1.T),  # (80, 8)
        "onesfold": t(of),
        "onesall": t(np.ones((128, 1))),
        "bc8": t(bc8),
        "ident": t(np.eye(128)),
    }


def _emit(nc, dbg=False):
    dt = F32
    dram_in = {}

    def din(name, shape):
        dram_in[name] = nc.dram_tensor(name, list(shape), dt, kind="ExternalInput").ap()
        return dram_in[name]

    xs = din("xs", [4, 128, 640])
    toepT = din("toepT", [128, 8 * 128])
    sgT = din("sg", [128, 10 * 128])
    wdT = din("wd", [128, 32])
    w2tT = din("w2t", [128, 16 * 128])
    fcwT = din("fcw", [128, 80])
    bonesT = din("bones", [128, 400])
    w1tT = din("w1t", [80, 8])
    ofT = din("onesfold", [128, 8])
    oaT = din("onesall", [128, 1])
    bc8T = din("bc8", [8, 128])
    idT = din("ident", [128, 128])
    g1T = din("g1", [8, 1])
    g2T = din("g2", [128, 1])
    b2T = din("b2", [128, 1])
    g3T = din("g3", [128, 1])
    b3T = din("b3", [128, 1])
    fcbT = din("fcb", [4, 1])

    out = nc.dram_tensor("out", [8, 4], dt, kind="ExternalOutput").ap()
    dbg_out = {}
    if dbg:
        for name, shape in [
            ("d_xw", [32, 2560]),
            ("d_xt", [128, 13 * 512]),
            ("d_g", [128, 5120]),
            ("d_gst", [128, 4]),
            ("d_h", [128, 1408]),
            ("d_s", [128, 1280]),
            ("d_h3", [128, 160]),
        ]:
            dbg_out[name] = nc.dram_tensor(name, shape, dt, kind="ExternalOutput").ap()

    fr = lambda ap: ap
    mdt = FR if _MM_DT == "f32r" else F32

    with tile.TileContext(nc) as tc:
        with (
            tc.tile_pool(name="const", bufs=1) as cp,
            tc.tile_pool(name="xbuf", bufs=1) as xp,
            tc.tile_pool(name="persist", bufs=1) as pp,
            tc.tile_pool(name="stats", bufs=1) as st,
            tc.tile_pool(name="scr", bufs=3) as scr,
            tc.tile_pool(name="dram", bufs=1, space="DRAM") as dp,
        ):
            # ---- constants ----
            c_toep = cp.tile([128, 1024], mdt, tag="toep")
            c_sg = cp.tile([128, 1280], mdt, tag="sg")
            c_wd = cp.tile([128, 32], mdt, tag="wd")
            c_w2t = cp.tile([128, 2048], mdt, tag="w2t")
            c_fcw = cp.tile([128, 80], dt, tag="fcw")
            c_bones = cp.tile([128, 400], dt, tag="bones")
            c_w1t = cp.tile([80, 8], dt, tag="w1t")
            c_of = cp.tile([128, 8], dt, tag="of")
            c_oa = cp.tile([128, 1], mdt, tag="oa")
            c_bc8 = cp.tile([8, 128], dt, tag="bc8")
            c_id = cp.tile([128, 128], mdt, tag="ident")
            c_g1 = cp.tile([8, 1], dt, tag="g1")
            c_g2 = cp.tile([128, 1], dt, tag="g2")
            c_b2 = cp.tile([128, 1], dt, tag="b2")
            c_g3 = cp.tile([128, 1], dt, tag="g3")
            c_b3 = cp.tile([128, 1], dt, tag="b3")
            c_fcb = cp.tile([4, 1], dt, tag="fcb")
            for t_, d_ in [
                (c_toep, toepT), (c_sg, sgT), (c_wd, wdT), (c_w2t, w2tT),
                (c_fcw, fcwT), (c_bones, bonesT), (c_w1t, w1tT), (c_of, ofT),
                (c_oa, oaT), (c_bc8, bc8T), (c_id, idT), (c_g1, g1T),
                (c_g2, g2T), (c_b2, b2T), (c_g3, g3T), (c_b3, b3T),
                (c_fcb, fcbT),
            ]:
                nc.gpsimd.dma_start(out=t_[:], in_=d_[:].bitcast(t_[:].dtype))

            # ---- x load: 4 tiles [(2b x 64n), 640] ----
            x_sb = xp.tile([128, 4 * 640], mdt, tag="x")
            for bp in range(4):
                nc.sync.dma_start(
                    out=x_sb[:, bp * 640 : (bp + 1) * 640], in_=xs[bp].bitcast(mdt)
                )

            # ---- phase A: projection xw, xs1 row-sum, xw2 replication ----
            xw_sb = pp.tile([32, 4 * 640], mdt, tag="xw")
            xs1_sb = pp.tile([1, 640], dt, tag="xs1")
            xs1t = pp.tile([128, 5], dt, tag="xs1t")
            u_sb = pp.tile([80, 1], dt, tag="u")
            pack = st.tile([128, 4], dt, tag="pack")
            nc.gpsimd.memset(pack[:], 0.0)
            with tc.tile_pool(name="ppA", bufs=2, space="PSUM") as ppA, \
                 tc.tile_pool(name="ppXS", bufs=1, space="PSUM") as ppXS:
                for bp in range(4):
                    for ch in range(2):
                        pxw = ppA.tile([32, 320], dt, tag="pxw")
                        sl = x_sb[:, bp * 640 + ch * 320 : bp * 640 + (ch + 1) * 320]
                        nc.tensor.matmul(
                            out=pxw[:], lhsT=fr(c_wd[:]), rhs=fr(sl),
                            start=True, stop=True,
                        )
                        nc.scalar.copy(
                            out=xw_sb[0:32, bp * 640 + ch * 320 : bp * 640 + (ch + 1) * 320],
                            in_=pxw[:],
                        )
                pxs = [ppXS.tile([1, 320], dt, tag=f"pxs{c}", name=f"pxs{c}") for c in range(2)]
                for bp in range(4):
                    for ch in range(2):
                        sl = x_sb[:, bp * 640 + ch * 320 : bp * 640 + (ch + 1) * 320]
                        nc.tensor.matmul(
                            out=pxs[ch][:], lhsT=fr(c_oa[:]), rhs=fr(sl),
                            start=(bp == 0), stop=(bp == 3),
                        )
                for ch in range(2):
                    nc.scalar.copy(
                        out=xs1_sb[0:1, ch * 320 : (ch + 1) * 320], in_=pxs[ch][:]
                    )
                # transpose xs1 -> [640 rows] in 5 chunks of 128
                for q in range(5):
                    ptq = ppA.tile([128, 1], dt, tag="small")
                    nc.tensor.transpose(
                        out=ptq[:],
                        in_=xs1_sb[0:1, q * 128 : (q + 1) * 128],
                        identity=c_id[0:1, 0:1].bitcast(F32),
                    )
                    nc.scalar.copy(out=xs1t[:, q : q + 1], in_=ptq[:])
                pu = ppA.tile([80, 1], dt, tag="small", name="pu")
                for q in range(5):
                    nc.tensor.matmul(
                        out=pu[:],
                        lhsT=fr(c_bones[:, q * 80 : (q + 1) * 80]),
                        rhs=fr(xs1t[:, q : q + 1]),
                        start=(q == 0), stop=(q == 4),
                    )
                nc.scalar.copy(out=u_sb[:], in_=pu[:])
                ps1 = ppA.tile([8, 1], dt, tag="small", name="ps1")
                nc.tensor.matmul(
                    out=ps1[:], lhsT=fr(c_w1t[:]), rhs=fr(u_sb[:]),
                    start=True, stop=True,
                )
                nc.scalar.copy(out=pack[0:8, 1:2], in_=ps1[:])

            # xw2: row (d*8+tau) col (b, tt) = xw_pad[d, b, tt - 40 + tau]
            xw2 = pp.tile([128, 8 * 720], mdt, tag="xw2")
            nc.gpsimd.memset(xw2[:].bitcast(F32), 0.0)
            xw2_v = xw2[:].rearrange("p (b tt) -> p b tt", b=8)
            xw_v = xw_sb[:].rearrange("(j d) (bp t) -> j d bp t", j=2, bp=4)
            for tau in range(8):
                for j in range(2):
                    nc.sync.dma_start(
                        out=xw2_v[tau : 128 : 8, j : 8 : 2, 40 - tau : 360 - tau],
                        in_=xw_v[j][:, :, 0:320],
                    )
            for tau in range(8):
                for j in range(2):
                    nc.sync.dma_start(
                        out=xw2_v[tau : 128 : 8, j : 8 : 2, 360 - tau : 680 - tau],
                        in_=xw_v[j][:, :, 320:640],
                    )
            if dbg:
                nc.sync.dma_start(out=dbg_out["d_xw"][:].bitcast(xw_sb[:].dtype), in_=xw_sb[:])

            if _PHASES < 2:
                return dram_in, out, dbg_out
            # ---- phase B: transposes + stats conv + G conv ----
            xt = xp.tile([128, 13 * 512], mdt, tag="xt")
            nc.gpsimd.memset(xt[:, 12 * 512 : 13 * 512].bitcast(F32), 0.0)
            q1c = st.tile([128, 40], dt, tag="q1c")
            s2c = st.tile([128, 10], dt, tag="s2c")
            q2c = st.tile([128, 10], dt, tag="q2c")
            g_sb = pp.tile([128, 5120], dt, tag="g")
            with tc.tile_pool(name="ppT", bufs=2, space="PSUM") as ppT, \
                 tc.tile_pool(name="ppY", bufs=4, space="PSUM") as ppY, \
                 tc.tile_pool(name="ppG", bufs=2, space="PSUM") as ppG:
                for i in range(13):
                    s = TILE_S[i]
                    w = min(128, 640 - s)
                    for bp in range(4):
                        pt = ppT.tile([128, 128], dt, tag="pt")
                        nc.tensor.transpose(
                            out=pt[0:w, :],
                            in_=x_sb[:, bp * 640 + s : bp * 640 + s + w].bitcast(F32),
                            identity=c_id[:].bitcast(F32),
                        )
                        dst = xt[0:w, i * 512 + bp * 128 : i * 512 + (bp + 1) * 128]
                        if (i * 4 + bp) % 2 == 0:
                            nc.scalar.copy(out=dst, in_=pt[0:w, :])
                        else:
                            nc.vector.tensor_copy(out=dst, in_=pt[0:w, :])
                if dbg:
                    nc.sync.dma_start(out=dbg_out["d_xt"][:].bitcast(xt[:].dtype), in_=xt[:])

                gi = 0
                for blk in range(40):
                    t0 = 16 * blk
                    i, p = _blk_tile_phi(t0)
                    py = ppY.tile([128, 512], dt, tag="py")
                    nc.tensor.matmul(
                        out=py[:],
                        lhsT=fr(c_toep[:, p * 128 : (p + 1) * 128]),
                        rhs=fr(xt[:, i * 512 : (i + 1) * 512]),
                        start=True, stop=True,
                    )
                    if blk % 3 != 1:
                        sa = scr.tile([128, 512], dt, tag="scrA")
                        nc.scalar.activation(
                            out=sa[:], in_=py[:], func=AF.Square,
                            accum_out=q1c[:, blk : blk + 1],
                        )
                    else:
                        # DVE cannot square from PSUM (one-PSUM-input rule):
                        # copy to SBUF on DVE, square+accum there.
                        sv = scr.tile([128, 512], dt, tag="scrV")
                        nc.vector.tensor_copy(out=sv[:], in_=py[:])
                        sv2 = scr.tile([128, 512], dt, tag="scrV2")
                        nc.vector.scalar_tensor_tensor(
                            out=sv2[:], in0=sv[:], scalar=0.0, in1=sv[:],
                            op0=AL.add, op1=AL.mult,
                            accum_out=q1c[:, blk : blk + 1],
                        )
                    # interleave G chunks (4 stats blocks : 1 G chunk)
                    if blk % 4 == 3 and gi < 10:
                        ch = gi
                        gi += 1
                        pg = ppG.tile([128, 512], dt, tag="pg")
                        for g in range(10):
                            tt0 = 64 * ch + 8 * g + 1
                            nc.tensor.matmul(
                                out=pg[:],
                                lhsT=fr(c_sg[:, g * 128 : (g + 1) * 128]),
                                rhs=fr(xw2_v[:, :, tt0 : tt0 + 64]),
                                start=(g == 0), stop=(g == 9),
                            )
                        nc.scalar.activation(
                            out=g_sb[:, ch * 512 : (ch + 1) * 512], in_=pg[:],
                            func=AF.Copy, accum_out=s2c[:, ch : ch + 1],
                        )
                        # square+accum from the SBUF copy of G (DVE).
                        gsl = g_sb[:, ch * 512 : (ch + 1) * 512]
                        sv2 = scr.tile([128, 512], dt, tag="scrV2")
                        nc.vector.scalar_tensor_tensor(
                            out=sv2[:], in0=gsl, scalar=0.0, in1=gsl,
                            op0=AL.add, op1=AL.mult,
                            accum_out=q2c[:, ch : ch + 1],
                        )
            if dbg:
                nc.sync.dma_start(out=dbg_out["d_g"][:], in_=g_sb[:])

            if _PHASES < 3:
                return dram_in, out, dbg_out
            # ---- stat folds + AllReduce 1 ----
            q1r = st.tile([128, 1], dt, tag="q1r")
            nc.vector.tensor_reduce(
                out=q1r[:], in_=q1c[:], axis=mybir.AxisListType.X, op=AL.add
            )
            nc.vector.tensor_reduce(
                out=pack[:, 2:3], in_=s2c[:], axis=mybir.AxisListType.X, op=AL.add
            )
            nc.vector.tensor_reduce(
                out=pack[:, 3:4], in_=q2c[:], axis=mybir.AxisListType.X, op=AL.add
            )
            with tc.tile_pool(name="ppF", bufs=2, space="PSUM") as ppF:
                p8 = ppF.tile([8, 1], dt, tag="p8")
                nc.tensor.matmul(
                    out=p8[:], lhsT=fr(c_of[:]), rhs=fr(q1r[:]),
                    start=True, stop=True,
                )
                nc.scalar.copy(out=pack[0:8, 0:1], in_=p8[:])

            ar_in = dp.tile([128, 4], dt, tag="arin")
            ar_out = dp.tile([128, 4], dt, tag="arout")
            nc.gpsimd.dma_start(out=ar_in[:], in_=pack[:])
            nc.gpsimd.collective_compute(
                "AllReduce", AL.add,
                replica_groups=[list(range(NCORES))],
                ins=[ar_in.opt()], outs=[ar_out.opt()],
            )
            gst = st.tile([128, 4], dt, tag="gst")
            nc.gpsimd.dma_start(out=gst[:], in_=ar_out[:])
            if dbg:
                nc.sync.dma_start(out=dbg_out["d_gst"][:], in_=gst[:])

            if _PHASES < 4:
                return dram_in, out, dbg_out
            # ---- post-AR math: per-channel affine A, B ----
            tA = lambda tag: st.tile([128, 1], dt, tag=tag, name=tag)
            v1 = tA("v1"); r1 = tA("r1"); a2 = tA("a2"); al = tA("al")
            ta = tA("ta"); tb = tA("tb"); tcda = tA("tc")
            # rows 0..7: v1 = Q1/BNT - (S1/BNT)^2
            nc.vector.tensor_scalar_mul(out=ta[0:8, :], in0=gst[0:8, 0:1], scalar1=1.0 / BNT)
            nc.vector.tensor_scalar_mul(out=tb[0:8, :], in0=gst[0:8, 1:2], scalar1=1.0 / BNT)
            nc.vector.tensor_tensor(out=tcda[0:8, :], in0=tb[0:8, :], in1=tb[0:8, :], op=AL.mult)
            nc.vector.tensor_sub(out=v1[0:8, :], in0=ta[0:8, :], in1=tcda[0:8, :])
            nc.vector.tensor_scalar_add(out=v1[0:8, :], in0=v1[0:8, :], scalar1=EPS)
            nc.vector.reciprocal(out=r1[0:8, :], in_=v1[0:8, :])
            # a2 = g1^2 * r1 ; al = g1 * sqrt(r1)
            nc.vector.tensor_tensor(out=ta[0:8, :], in0=c_g1[:], in1=c_g1[:], op=AL.mult)
            nc.vector.tensor_tensor(out=a2[0:8, :], in0=ta[0:8, :], in1=r1[0:8, :], op=AL.mult)
            nc.scalar.activation(out=tb[0:8, :], in_=r1[0:8, :], func=AF.Sqrt)
            nc.vector.tensor_tensor(out=al[0:8, :], in0=c_g1[:], in1=tb[0:8, :], op=AL.mult)
            # broadcast to 128 channels via tiny matmuls
            a2c = tA("a2c"); alc = tA("alc")
            with tc.tile_pool(name="ppB", bufs=2, space="PSUM") as ppB:
                pb1 = ppB.tile([128, 1], dt, tag="pb1")
                nc.tensor.matmul(out=pb1[:], lhsT=fr(c_bc8[:]), rhs=fr(a2[0:8, :]), start=True, stop=True)
                nc.scalar.copy(out=a2c[:], in_=pb1[:])
                pb2 = ppB.tile([128, 1], dt, tag="pb1")
                nc.tensor.matmul(out=pb2[:], lhsT=fr(c_bc8[:]), rhs=fr(al[0:8, :]), start=True, stop=True)
                nc.scalar.copy(out=alc[:], in_=pb2[:])
            mg = tA("mg"); vg = tA("vg"); r2 = tA("r2"); Aff = tA("Aff"); Bff = tA("Bff")
            nc.vector.tensor_scalar_mul(out=mg[:], in0=gst[:, 2:3], scalar1=1.0 / BT)
            nc.vector.tensor_scalar_mul(out=vg[:], in0=gst[:, 3:4], scalar1=1.0 / BT)
            nc.vector.tensor_tensor(out=ta[:], in0=mg[:], in1=mg[:], op=AL.mult)
            nc.vector.tensor_sub(out=vg[:], in0=vg[:], in1=ta[:])
            nc.vector.tensor_tensor(out=vg[:], in0=vg[:], in1=a2c[:], op=AL.mult)
            nc.vector.tensor_scalar_add(out=vg[:], in0=vg[:], scalar1=EPS)
            nc.vector.reciprocal(out=r2[:], in_=vg[:])
            nc.scalar.activation(out=ta[:], in_=r2[:], func=AF.Sqrt)
            nc.vector.tensor_tensor(out=tb[:], in0=alc[:], in1=ta[:], op=AL.mult)
            nc.vector.tensor_tensor(out=Aff[:], in0=tb[:], in1=c_g2[:], op=AL.mult)
            nc.vector.tensor_tensor(out=ta[:], in0=Aff[:], in1=mg[:], op=AL.mult)
            nc.vector.tensor_sub(out=Bff[:], in0=c_b2[:], in1=ta[:])

            if _PHASES < 5:
                return dram_in, out, dbg_out
            # ---- affine + ELU + pool4 -> h_pad ----
            h_pad = pp.tile([128, 8 * 176], mdt, tag="hpad")
            nc.gpsimd.memset(h_pad[:].bitcast(F32), 0.0)
            with tc.tile_pool(name="elu", bufs=1) as ep:
                u_t = ep.tile([128, 5120], dt, tag="ut")
                e_t = ep.tile([128, 5120], dt, tag="et")
                pl1 = ep.tile([128, 2560], dt, tag="pl1")
                u_v = u_t[:].rearrange("p (c b t) -> p c b t", c=10, b=8)
                g_v = g_sb[:].rearrange("p (c b t) -> p c b t", c=10, b=8)
                e_v = e_t[:].rearrange("p (c b t) -> p c b t", c=10, b=8)
                p1_v = pl1[:].rearrange("p (c b t) -> p c b t", c=10, b=8)
                h_v = (
                    h_pad[:]
                    .rearrange("p (b w) -> p b w", b=8)[:, :, 7:167]
                    .rearrange("p b (c tl) -> p c b tl", c=10)
                )
                # per b-pair: affine+ELU+pool (ACT/DVE/GpSimd), handing each
                # pair to the sep conv (PE) as soon as it lands in h_pad
                for cb in range(4):
                    bs = slice(2 * cb, 2 * cb + 2)
                    nc.scalar.activation(
                        out=u_v[:, :, bs], in_=g_v[:, :, bs], func=AF.Identity,
                        scale=Aff[:, 0:1], bias=Bff[:, 0:1],
                    )
                    nc.vector.tensor_scalar_min(
                        out=g_v[:, :, bs], in0=u_v[:, :, bs], scalar1=0.0
                    )
                    nc.scalar.activation(
                        out=e_v[:, :, bs], in_=g_v[:, :, bs], func=AF.Exp
                    )
                    nc.vector.tensor_scalar_max(
                        out=u_v[:, :, bs], in0=u_v[:, :, bs], scalar1=0.0
                    )
                    nc.vector.scalar_tensor_tensor(
                        out=u_v[:, :, bs], in0=e_v[:, :, bs], scalar=-1.0,
                        in1=u_v[:, :, bs], op0=AL.add, op1=AL.add,
                    )
                    nc.gpsimd.tensor_add(
                        out=p1_v[:, :, bs], in0=u_v[:, :, bs, 0:64:2],
                        in1=u_v[:, :, bs, 1:64:2],
                    )
                    nc.vector.tensor_add(
                        out=h_v[:, :, bs], in0=p1_v[:, :, bs, 0:32:2],
                        in1=p1_v[:, :, bs, 1:32:2],
                    )
            if dbg:
                nc.sync.dma_start(out=dbg_out["d_h"][:].bitcast(h_pad[:].dtype), in_=h_pad[:])

            if _PHASES < 6:
                return dram_in, out, dbg_out
            # ---- sep conv + BN3 stats ----
            s_sb = pp.tile([128, 1280], dt, tag="ssb")
            q3c = st.tile([128, 4], dt, tag="q3c")
            pack2 = st.tile([128, 2], dt, tag="pack2")
            h_pv = h_pad[:].rearrange("p (b w) -> p b w", b=8)
            with tc.tile_pool(name="ppS", bufs=1, space="PSUM") as ppS:
                psb = [ppS.tile([128, 320], dt, tag=f"ps{cb}", name=f"ps{cb}") for cb in range(4)]
                # cb-outer so each b-pair's taps start as soon as its
                # h_pad slice is pooled
                for cb in range(4):
                    for tap in range(16):
                        rhs = h_pv[:, 2 * cb : 2 * cb + 2, tap : tap + 160]
                        nc.tensor.matmul(
                            out=psb[cb][:],
                            lhsT=fr(c_w2t[:, tap * 128 : (tap + 1) * 128]),
                            rhs=fr(rhs),
                            start=(tap == 0), stop=(tap == 15),
                        )
                s3tmp = st.tile([128, 4], dt, tag="s3tmp")
                for cb in range(4):
                    nc.scalar.activation(
                        out=s_sb[:, cb * 320 : (cb + 1) * 320], in_=psb[cb][:],
                        func=AF.Copy, accum_out=s3tmp[:, cb : cb + 1],
                    )
                    ssl = s_sb[:, cb * 320 : (cb + 1) * 320]
                    sv3 = scr.tile([128, 512], dt, tag="scrV2")
                    nc.vector.scalar_tensor_tensor(
                        out=sv3[:, 0:320], in0=ssl, scalar=0.0, in1=ssl,
                        op0=AL.add, op1=AL.mult,
                        accum_out=q3c[:, cb : cb + 1],
                    )
            nc.vector.tensor_reduce(
                out=pack2[:, 0:1], in_=s3tmp[:], axis=mybir.AxisListType.X, op=AL.add
            )
            nc.vector.tensor_reduce(
                out=pack2[:, 1:2], in_=q3c[:], axis=mybir.AxisListType.X, op=AL.add
            )
            if dbg:
                nc.sync.dma_start(out=dbg_out["d_s"][:], in_=s_sb[:])

            ar2_in = dp.tile([128, 2], dt, tag="ar2in")
            ar2_out = dp.tile([128, 2], dt, tag="ar2out")
            nc.gpsimd.dma_start(out=ar2_in[:], in_=pack2[:])
            nc.gpsimd.collective_compute(
                "AllReduce", AL.add,
                replica_groups=[list(range(NCORES))],
                ins=[ar2_in.opt()], outs=[ar2_out.opt()],
            )
            gst2 = st.tile([128, 2], dt, tag="gst2")
            nc.gpsimd.dma_start(out=gst2[:], in_=ar2_out[:])

            m3 = tA("m3"); v3 = tA("v3"); A3 = tA("A3"); B3 = tA("B3")
            nc.vector.tensor_scalar_mul(out=m3[:], in0=gst2[:, 0:1], scalar1=1.0 / BT2)
            nc.vector.tensor_scalar_mul(out=v3[:], in0=gst2[:, 1:2], scalar1=1.0 / BT2)
            nc.vector.tensor_tensor(out=ta[:], in0=m3[:], in1=m3[:], op=AL.mult)
            nc.vector.tensor_sub(out=v3[:], in0=v3[:], in1=ta[:])
            nc.vector.tensor_scalar_add(out=v3[:], in0=v3[:], scalar1=EPS)
            nc.vector.reciprocal(out=ta[:], in_=v3[:])
            nc.scalar.activation(out=tb[:], in_=ta[:], func=AF.Sqrt)
            nc.vector.tensor_tensor(out=A3[:], in0=tb[:], in1=c_g3[:], op=AL.mult)
            nc.vector.tensor_tensor(out=ta[:], in0=A3[:], in1=m3[:], op=AL.mult)
            nc.vector.tensor_sub(out=B3[:], in0=c_b3[:], in1=ta[:])

            if _PHASES < 7:
                return dram_in, out, dbg_out
            # ---- BN3 affine + ELU + pool8 -> h3 ----
            h3 = pp.tile([128, 160], dt, tag="h3")
            with tc.tile_pool(name="elu3", bufs=1) as e3p:
                v3t = e3p.tile([128, 1280], dt, tag="v3t")
                e3t = e3p.tile([128, 1280], dt, tag="e3t")
                nc.scalar.activation(
                    out=v3t[:], in_=s_sb[:], func=AF.Identity,
                    scale=A3[:, 0:1], bias=B3[:, 0:1],
                )
                nc.vector.tensor_scalar_min(out=s_sb[:], in0=v3t[:], scalar1=0.0)
                nc.scalar.activation(out=e3t[:], in_=s_sb[:], func=AF.Exp)
                nc.vector.tensor_scalar_max(out=v3t[:], in0=v3t[:], scalar1=0.0)
                nc.vector.scalar_tensor_tensor(
                    out=v3t[:], in0=e3t[:], scalar=-1.0, in1=v3t[:],
                    op0=AL.add, op1=AL.add,
                )
                pq1 = e3p.tile([128, 640], dt, tag="pq1")
                pq2 = e3p.tile([128, 320], dt, tag="pq2")
                nc.gpsimd.tensor_add(
                    out=pq1[:], in0=v3t[:, 0:1280:2], in1=v3t[:, 1:1280:2]
                )
                nc.vector.tensor_add(
                    out=pq2[:], in0=pq1[:, 0:640:2], in1=pq1[:, 1:640:2]
                )
                nc.gpsimd.tensor_add(
                    out=h3[:], in0=pq2[:, 0:320:2], in1=pq2[:, 1:320:2]
                )
            if dbg:
                nc.sync.dma_start(out=dbg_out["d_h3"][:], in_=h3[:])

            if _PHASES < 8:
                return dram_in, out, dbg_out
            # ---- FC ----
            h3_v = h3[:].rearrange("p (b t) -> p b t", b=8)
            o_sb = st.tile([4, 8], dt, tag="osb")
            with tc.tile_pool(name="ppO", bufs=1, space="PSUM") as ppO:
                po = ppO.tile([4, 8], dt, tag="po")
                for t2 in range(20):
                    nc.tensor.matmul(
                        out=po[:],
                        lhsT=fr(c_fcw[:, t2 * 4 : (t2 + 1) * 4]),
                        rhs=fr(h3_v[:, :, t2]),
                        start=(t2 == 0), stop=(t2 == 19),
                    )
                nc.scalar.copy(out=o_sb[:], in_=po[:])
            nc.vector.tensor_scalar_add(
                out=o_sb[:], in0=o_sb[:], scalar1=c_fcb[0:4, 0:1]
            )
            nc.sync.dma_start(out=out.rearrange("b j -> j b"), in_=o_sb[:])

    return dram_in, out, dbg_out


def build_and_run(inputs, dbg=False, trace=False):
    _patch_tile_tail()
    X = np.asarray(inputs["X"], np.float32)
    consts = _host_tensors(
        inputs["L"], inputs["conv1_w"], inputs["cheb_w"],
        inputs["sep_w"], inputs["fc_w"],
    )
    common = dict(consts)
    common["g1"] = np.ascontiguousarray(np.asarray(inputs["bn1_g"], np.float32).reshape(8, 1))
    common["g2"] = np.ascontiguousarray(np.asarray(inputs["bn2_g"], np.float32).reshape(128, 1))
    common["b2"] = np.ascontiguousarray(np.asarray(inputs["bn2_b"], np.float32).reshape(128, 1))
    common["g3"] = np.ascontiguousarray(np.asarray(inputs["bn3_g"], np.float32).reshape(128, 1))
    common["b3"] = np.ascontiguousarray(np.asarray(inputs["bn3_b"], np.float32).reshape(128, 1))
    common["fcb"] = np.ascontiguousarray(np.asarray(inputs["fc_b"], np.float32).reshape(4, 1))

    nc = bass.Bass("TRN2", target_bir_lowering=False)
    _emit(nc, dbg=dbg)
    _split_waits(nc)

    in_maps = []
    for core in range(NCORES):
        m = dict(common)
        m["xs"] = np.ascontiguousarray(
            X[core * BL : (core + 1) * BL].reshape(4, 128, 640)
        )
        in_maps.append(m)
    res = run_bass_kernel_spmd(nc, in_maps, list(range(NCORES)), trace=trace)
    out = np.concatenate([res.results[i]["out"] for i in range(NCORES)], axis=0)
    return out.astype(np.float32), res


def kernel(**inputs):
    out, _ = build_and_run(inputs)
    return out


# revision 22
# speedup vs baseline: 1.2674x; 1.2674x over previous
"""EEGGCNet Trainium2 kernel: 8-core batch-parallel Bass/Tile implementation.

Pipeline (per core, local batch of 8):
  conv1 (temporal, 80-tap) -> BN1 -> Chebyshev graph conv + node-mean ->
  BN2 -> ELU -> pool4 -> separable conv (128ch, 16-tap) -> BN3 -> ELU ->
  pool8 -> FC.

Algebraic restructuring (all exact):
  * The Chebyshev recursion + node-mean collapses to a single (16 x 64)
    projection W applied over the node axis (host-precomputed from L and
    cheb_w; polynomials of L commute with L).
  * BN affine constants (conv1_b, bn1_b, cheb_b, sep_b) cancel inside the
    mean-subtractions; only bn variances and means of the *raw* conv
    outputs are needed.
  * BatchNorm batch statistics are reduced across cores with two tiny
    AllReduces (sums + sums-of-squares).
  * avgpool divisors are folded into downstream weights.

Layouts:
  * conv1-for-stats runs time-major via PE transposes + banded-Toeplitz
    stationary matrices (host-built from conv1_w).
  * The projected conv (G path) runs channel-major using an 8x shifted
    replication of the projected signal xw; contraction dim = (d, tau).
  * sep conv is a plain 16-tap accumulation matmul over channels.
"""

import sys

sys.path.insert(0, "/opt/trn_rl_repo")

import numpy as np

import concourse.bass as bass
import concourse.mybir as mybir
from concourse import tile
from concourse.vector_clock import ScopedClock
from concourse.bass_utils import run_bass_kernel_spmd

F1, D, K, CH, T, KL, NCLS = 8, 16, 5, 64, 640, 80, 4
B, NCORES, BL = 64, 8, 8
EPS = 1e-5
T2, T3 = 160, 20
BNT = float(B * CH * T)
BT = float(B * T)
BT2 = float(B * T2)
F32 = mybir.dt.float32
FR = mybir.dt.float32r
AL = mybir.AluOpType
AF = mybir.ActivationFunctionType
_MM_DT = "f32r"  # "f32" | "f32r"
_PHASES = 99  # emission truncation for bisection

# Tile starts for the time-major stats conv: tile 0 at t=0 holds output
# blocks t0 in {0,16,32,48,64}; tiles i=1..12 at s=41+48(i-1) hold blocks
# s+{39,55,71}.  8 distinct Toeplitz phases.
PHI = [0, 16, 32, 48, 64, 39, 55, 71]
TILE_S = [0] + [41 + 48 * (i - 1) for i in range(1, 13)]


def _blk_tile_phi(t0):
    if t0 <= 64:
        return 0, PHI.index(t0)
    i = (t0 - 80) // 48 + 1
    return i, PHI.index(t0 - TILE_S[i])


def _patch_tile_tail():
    """This walrus build allows at most ~2 sync waits on TPB_CTRL-class
    instructions (Drain) and rejects sem-eq-imm waits on them.  Split the
    TileContext tail-drain waits one-per-nop and use the sem-only
    barrier."""

    def _drain_and_barrier(self, tick_clock, wait_clock):
        nc = self.nc
        probe = nc.sync.nop(nofuse=True)
        wait_clock.add_sem_waits(
            probe.ins, ScopedClock({None: tick_clock.global_clock})
        )
        si = probe.ins.sync_info
        waits = list(si.on_wait or []) if si is not None else []
        if si is not None:
            si.on_wait = waits[:1]
        for w in waits[1:]:
            n2 = nc.sync.nop(nofuse=True)
            if n2.ins.sync_info is None:
                n2.ins.sync_info = mybir.SyncInfo(on_wait=[w], on_update=[])
            else:
                n2.ins.sync_info.on_wait = [w]
        nc.sync.drain()
        nc.all_engine_barrier(sem_only=True)
        popped = nc._tile_sem_poison_stack.pop()
        assert popped is self._sem_poison
        sems = list(self.sems.allocated().values())
        for i in range(0, len(sems), 8):
            nc.clear_and_free_semaphores(sems[i : i + 8])
        nc.all_engine_barrier(sem_only=True)

    tile.TileContext._drain_and_barrier = _drain_and_barrier


def _split_waits(nc, limit=1):
    """This walrus build supports very few sync-wait slots per instruction.
    Hoist excess waits onto same-engine nops placed immediately before the
    instruction (engine queues are FIFO, so semantics are preserved)."""
    ctr = [0]
    for bb in nc.main_func.blocks:
        new = []
        for ins in bb.instructions:
            si = ins.sync_info
            waits = list(si.on_wait) if (si is not None and si.on_wait) else []
            if len(waits) > limit:
                for w in waits[:-limit]:
                    ctr[0] += 1
                    nop = mybir.InstNoOp(name=f"wsplit_{ctr[0]}", ins=[], outs=[])
                    nop.engine = ins.engine
                    nop.sync_info = mybir.SyncInfo(on_wait=[w], on_update=[])
                    nc.register_instruction(nop, overwrite=True)
                    new.append(nop)
                si.on_wait = waits[-limit:]
            new.append(ins)
        bb.instructions[:] = new


def _host_tensors(L, conv1_w, cheb_w, sep_w, fc_w):
    w1 = np.asarray(conv1_w, np.float64)[:, 0, 0, :]  # (8, 80)
    Lm = np.asarray(L, np.float64)

    # Chebyshev node-mean row vectors: V[k] = (1/N) 1^T P_k(L)
    V = np.zeros((K, CH))
    V[0] = 1.0 / CH
    V[1] = V[0] @ Lm
    for k in range(2, K):
        V[k] = 2.0 * (V[k - 1] @ Lm) - V[k - 2]
    W = np.asarray(cheb_w, np.float64)[:, 0, :].T @ V  # (16, 64)

    # Block-diag projection stationary (two batches per matmul).
    wd = np.zeros((128, 32))
    for j in range(2):
        wd[j * 64 : (j + 1) * 64, j * 16 : (j + 1) * 16] = W.T

    # Toeplitz stationaries for the time-major stats conv.
    toep = np.zeros((128, 8 * 128))
    for p, phi in enumerate(PHI):
        for f in range(F1):
            for j in range(16):
                for r in range(128):
                    tap = r - j - phi + 39
                    if 0 <= tap < KL:
                        toep[r, p * 128 + f * 16 + j] = w1[f, tap]

    # Channel-major G-conv stationaries: k = (d, tau8), m = (f, d).
    sg = np.zeros((128, 10 * 128))
    for g in range(10):
        for d in range(D):
            for tau in range(8):
                for f in range(F1):
                    sg[d * 8 + tau, g * 128 + f * 16 + d] = w1[f, 8 * g + tau]

    # sep conv stationaries per tap, pool4 divisor folded in.
    w2t = np.zeros((128, 16 * 128))
    sw = np.asarray(sep_w, np.float64)[:, :, 0, :]  # (oc, ic, tap)
    for tap in range(16):
        w2t[:, tap * 128 : (tap + 1) * 128] = sw[:, :, tap].T / 4.0

    # FC stationaries per pooled-time index, pool8 divisor folded in.
    fcw = np.zeros((128, 20 * 4))
    fw = np.asarray(fc_w, np.float64)  # (4, 2560), flat = c*20 + t''
    for t2 in range(20):
        for j in range(NCLS):
            fcw[:, t2 * 4 + j] = fw[j, t2::20] / 8.0

    # Banded ones for the windowed sums u[tau] (sum_y path).
    bones = np.zeros((128, 5 * 80))
    for q in range(5):
        for r in range(128):
            s = q * 128 + r
            for tau in range(KL):
                if tau - 39 <= s <= 600 + tau:
                    bones[r, q * 80 + tau] = 1.0

    of = np.zeros((128, 8))
    for f in range(F1):
        of[f * 16 : (f + 1) * 16, f] = 1.0
    bc8 = np.zeros((8, 128))
    for f in range(F1):
        bc8[f, f * 16 : (f + 1) * 16] = 1.0

    t = lambda a: np.ascontiguousarray(a, np.float32)
    return {
        "toepT": t(toep),
        "sg": t(sg),
        "wd": t(wd),
        "w2t": t(w2t),
        "fcw": t(fcw),
        "bones": t(bones),
        "w1t": t(w1.T),  # (80, 8)
        "onesfold": t(of),
        "onesall": t(np.ones((128, 1))),
        "bc8": t(bc8),
        "ident": t(np.eye(128)),
    }


def _emit(nc, dbg=False):
    dt = F32
    dram_in = {}

    def din(name, shape):
        dram_in[name] = nc.dram_tensor(name, list(shape), dt, kind="ExternalInput").ap()
        return dram_in[name]

    xs = din("xs", [4, 128, 640])
    toepT = din("toepT", [128, 8 * 128])
    sgT = din("sg", [128, 10 * 128])
    wdT = din("wd", [128, 32])
    w2tT = din("w2t", [128, 16 * 128])
    fcwT = din("fcw", [128, 80])
    bonesT = din("bones", [128, 400])
    w1tT = din("w1t", [80, 8])
    ofT = din("onesfold", [128, 8])
    oaT = din("onesall", [128, 1])
    bc8T = din("bc8", [8, 128])
    idT = din("ident", [128, 128])
    g1T = din("g1", [8, 1])
    g2T = din("g2", [128, 1])
    b2T = din("b2", [128, 1])
    g3T = din("g3", [128, 1])
    b3T = din("b3", [128, 1])
    fcbT = din("fcb", [4, 1])

    out = nc.dram_tensor("out", [8, 4], dt, kind="ExternalOutput").ap()
    dbg_out = {}
    if dbg:
        for name, shape in [
            ("d_xw", [32, 2560]),
            ("d_xt", [128, 13 * 512]),
            ("d_g", [128, 5120]),
            ("d_gst", [128, 4]),
            ("d_h", [128, 1408]),
            ("d_s", [128, 1280]),
            ("d_h3", [128, 160]),
        ]:
            dbg_out[name] = nc.dram_tensor(name, shape, dt, kind="ExternalOutput").ap()

    fr = lambda ap: ap
    mdt = FR if _MM_DT == "f32r" else F32

    with tile.TileContext(nc) as tc:
        with (
            tc.tile_pool(name="const", bufs=1) as cp,
            tc.tile_pool(name="xbuf", bufs=1) as xp,
            tc.tile_pool(name="persist", bufs=1) as pp,
            tc.tile_pool(name="stats", bufs=1) as st,
            tc.tile_pool(name="scr", bufs=3) as scr,
            tc.tile_pool(name="dram", bufs=1, space="DRAM") as dp,
        ):
            # ---- constants ----
            c_toep = cp.tile([128, 1024], mdt, tag="toep")
            c_sg = cp.tile([128, 1280], mdt, tag="sg")
            c_wd = cp.tile([128, 32], mdt, tag="wd")
            c_w2t = cp.tile([128, 2048], mdt, tag="w2t")
            c_fcw = cp.tile([128, 80], dt, tag="fcw")
            c_bones = cp.tile([128, 400], dt, tag="bones")
            c_w1t = cp.tile([80, 8], dt, tag="w1t")
            c_of = cp.tile([128, 8], dt, tag="of")
            c_oa = cp.tile([128, 1], mdt, tag="oa")
            c_bc8 = cp.tile([8, 128], dt, tag="bc8")
            c_id = cp.tile([128, 128], mdt, tag="ident")
            c_g1 = cp.tile([8, 1], dt, tag="g1")
            c_g2 = cp.tile([128, 1], dt, tag="g2")
            c_b2 = cp.tile([128, 1], dt, tag="b2")
            c_g3 = cp.tile([128, 1], dt, tag="g3")
            c_b3 = cp.tile([128, 1], dt, tag="b3")
            c_fcb = cp.tile([4, 1], dt, tag="fcb")
            # x first (transposes gate on it), then matmul-critical weights
            # on the HWDGE queue in consumption order; small/late tensors on
            # SWDGE so they don't block the critical path.
            pass
            # ---- x load: 4 tiles [(2b x 64n), 640] ----
            x_sb = xp.tile([128, 4 * 640], mdt, tag="x")
            for bp in range(4):
                nc.sync.dma_start(
                    out=x_sb[:, bp * 640 : (bp + 1) * 640], in_=xs[bp].bitcast(mdt)
                )
            for t_, d_ in [(c_id, idT), (c_toep, toepT), (c_wd, wdT), (c_oa, oaT), (c_sg, sgT)]:
                nc.sync.dma_start(out=t_[:], in_=d_[:].bitcast(t_[:].dtype))
            for t_, d_ in [
                (c_w2t, w2tT), (c_fcw, fcwT), (c_bones, bonesT), (c_w1t, w1tT),
                (c_of, ofT), (c_bc8, bc8T), (c_g1, g1T), (c_g2, g2T),
                (c_b2, b2T), (c_g3, g3T), (c_b3, b3T), (c_fcb, fcbT),
            ]:
                nc.gpsimd.dma_start(out=t_[:], in_=d_[:].bitcast(t_[:].dtype))

            # ---- phase A: projection xw, xs1 row-sum, xw2 replication ----
            xw_sb = pp.tile([32, 4 * 640], mdt, tag="xw")
            xs1_sb = pp.tile([1, 640], dt, tag="xs1")
            xs1t = pp.tile([128, 5], dt, tag="xs1t")
            u_sb = pp.tile([80, 1], dt, tag="u")
            pack = st.tile([128, 4], dt, tag="pack")
            nc.gpsimd.memset(pack[:], 0.0)
            with tc.tile_pool(name="ppA", bufs=2, space="PSUM") as ppA, \
                 tc.tile_pool(name="ppXS", bufs=1, space="PSUM") as ppXS:
                for bp in range(4):
                    for ch in range(2):
                        pxw = ppA.tile([32, 320], dt, tag="pxw")
                        sl = x_sb[:, bp * 640 + ch * 320 : bp * 640 + (ch + 1) * 320]
                        nc.tensor.matmul(
                            out=pxw[:], lhsT=fr(c_wd[:]), rhs=fr(sl),
                            start=True, stop=True,
                        )
                        nc.scalar.copy(
                            out=xw_sb[0:32, bp * 640 + ch * 320 : bp * 640 + (ch + 1) * 320],
                            in_=pxw[:],
                        )
                pxs = [ppXS.tile([1, 320], dt, tag=f"pxs{c}", name=f"pxs{c}") for c in range(2)]
                for bp in range(4):
                    for ch in range(2):
                        sl = x_sb[:, bp * 640 + ch * 320 : bp * 640 + (ch + 1) * 320]
                        nc.tensor.matmul(
                            out=pxs[ch][:], lhsT=fr(c_oa[:]), rhs=fr(sl),
                            start=(bp == 0), stop=(bp == 3),
                        )
                for ch in range(2):
                    nc.scalar.copy(
                        out=xs1_sb[0:1, ch * 320 : (ch + 1) * 320], in_=pxs[ch][:]
                    )
                # transpose xs1 -> [640 rows] in 5 chunks of 128
                for q in range(5):
                    ptq = ppA.tile([128, 1], dt, tag="small")
                    nc.tensor.transpose(
                        out=ptq[:],
                        in_=xs1_sb[0:1, q * 128 : (q + 1) * 128],
                        identity=c_id[0:1, 0:1].bitcast(F32),
                    )
                    nc.scalar.copy(out=xs1t[:, q : q + 1], in_=ptq[:])
                pu = ppA.tile([80, 1], dt, tag="small", name="pu")
                for q in range(5):
                    nc.tensor.matmul(
                        out=pu[:],
                        lhsT=fr(c_bones[:, q * 80 : (q + 1) * 80]),
                        rhs=fr(xs1t[:, q : q + 1]),
                        start=(q == 0), stop=(q == 4),
                    )
                nc.scalar.copy(out=u_sb[:], in_=pu[:])
                ps1 = ppA.tile([8, 1], dt, tag="small", name="ps1")
                nc.tensor.matmul(
                    out=ps1[:], lhsT=fr(c_w1t[:]), rhs=fr(u_sb[:]),
                    start=True, stop=True,
                )
                nc.scalar.copy(out=pack[0:8, 1:2], in_=ps1[:])

            # xw2: row (d*8+tau) col (b, tt) = xw_pad[d, b, tt - 40 + tau]
            xw2 = pp.tile([128, 8 * 720], mdt, tag="xw2")
            nc.gpsimd.memset(xw2[:].bitcast(F32), 0.0)
            xw2_v = xw2[:].rearrange("p (b tt) -> p b tt", b=8)
            xw_v = xw_sb[:].rearrange("(j d) (bp t) -> j d bp t", j=2, bp=4)
            for tau in range(8):
                for j in range(2):
                    nc.sync.dma_start(
                        out=xw2_v[tau : 128 : 8, j : 8 : 2, 40 - tau : 360 - tau],
                        in_=xw_v[j][:, :, 0:320],
                    )
            for tau in range(8):
                for j in range(2):
                    nc.sync.dma_start(
                        out=xw2_v[tau : 128 : 8, j : 8 : 2, 360 - tau : 680 - tau],
                        in_=xw_v[j][:, :, 320:640],
                    )
            if dbg:
                nc.sync.dma_start(out=dbg_out["d_xw"][:].bitcast(xw_sb[:].dtype), in_=xw_sb[:])

            if _PHASES < 2:
                return dram_in, out, dbg_out
            # ---- phase B: transposes + stats conv + G conv ----
            xt = xp.tile([128, 13 * 512], mdt, tag="xt")
            nc.gpsimd.memset(xt[:, 12 * 512 : 13 * 512].bitcast(F32), 0.0)
            q1c = st.tile([128, 40], dt, tag="q1c")
            s2c = st.tile([128, 10], dt, tag="s2c")
            q2c = st.tile([128, 10], dt, tag="q2c")
            g_sb = pp.tile([128, 5120], dt, tag="g")
            with tc.tile_pool(name="ppT", bufs=2, space="PSUM") as ppT, \
                 tc.tile_pool(name="ppY", bufs=4, space="PSUM") as ppY, \
                 tc.tile_pool(name="ppG", bufs=2, space="PSUM") as ppG:
                for i in range(13):
                    s = TILE_S[i]
                    w = min(128, 640 - s)
                    for bp in range(4):
                        pt = ppT.tile([128, 128], dt, tag="pt")
                        nc.tensor.transpose(
                            out=pt[0:w, :],
                            in_=x_sb[:, bp * 640 + s : bp * 640 + s + w].bitcast(F32),
                            identity=c_id[:].bitcast(F32),
                        )
                        dst = xt[0:w, i * 512 + bp * 128 : i * 512 + (bp + 1) * 128]
                        if (i * 4 + bp) % 2 == 0:
                            nc.scalar.copy(out=dst, in_=pt[0:w, :])
                        else:
                            nc.vector.tensor_copy(out=dst, in_=pt[0:w, :])
                if dbg:
                    nc.sync.dma_start(out=dbg_out["d_xt"][:].bitcast(xt[:].dtype), in_=xt[:])

                gi = 0
                for blk in range(40):
                    t0 = 16 * blk
                    i, p = _blk_tile_phi(t0)
                    py = ppY.tile([128, 512], dt, tag="py")
                    nc.tensor.matmul(
                        out=py[:],
                        lhsT=fr(c_toep[:, p * 128 : (p + 1) * 128]),
                        rhs=fr(xt[:, i * 512 : (i + 1) * 512]),
                        start=True, stop=True,
                    )
                    if blk % 3 != 1:
                        sa = scr.tile([128, 512], dt, tag="scrA")
                        nc.scalar.activation(
                            out=sa[:], in_=py[:], func=AF.Square,
                            accum_out=q1c[:, blk : blk + 1],
                        )
                    else:
                        # DVE cannot square from PSUM (one-PSUM-input rule):
                        # copy to SBUF on DVE, square+accum there.
                        sv = scr.tile([128, 512], dt, tag="scrV")
                        nc.vector.tensor_copy(out=sv[:], in_=py[:])
                        sv2 = scr.tile([128, 512], dt, tag="scrV2")
                        nc.vector.scalar_tensor_tensor(
                            out=sv2[:], in0=sv[:], scalar=0.0, in1=sv[:],
                            op0=AL.add, op1=AL.mult,
                            accum_out=q1c[:, blk : blk + 1],
                        )
                    # interleave G chunks (4 stats blocks : 1 G chunk)
                    if blk % 4 == 3 and gi < 10:
                        ch = gi
                        gi += 1
                        pg = ppG.tile([128, 512], dt, tag="pg")
                        for g in range(10):
                            tt0 = 64 * ch + 8 * g + 1
                            nc.tensor.matmul(
                                out=pg[:],
                                lhsT=fr(c_sg[:, g * 128 : (g + 1) * 128]),
                                rhs=fr(xw2_v[:, :, tt0 : tt0 + 64]),
                                start=(g == 0), stop=(g == 9),
                            )
                        nc.scalar.activation(
                            out=g_sb[:, ch * 512 : (ch + 1) * 512], in_=pg[:],
                            func=AF.Copy, accum_out=s2c[:, ch : ch + 1],
                        )
                        # square+accum from the SBUF copy of G (DVE).
                        gsl = g_sb[:, ch * 512 : (ch + 1) * 512]
                        sv2 = scr.tile([128, 512], dt, tag="scrV2")
                        nc.vector.scalar_tensor_tensor(
                            out=sv2[:], in0=gsl, scalar=0.0, in1=gsl,
                            op0=AL.add, op1=AL.mult,
                            accum_out=q2c[:, ch : ch + 1],
                        )
            if dbg:
                nc.sync.dma_start(out=dbg_out["d_g"][:], in_=g_sb[:])

            if _PHASES < 3:
                return dram_in, out, dbg_out
            # ---- stat folds + AllReduce 1 ----
            q1r = st.tile([128, 1], dt, tag="q1r")
            nc.vector.tensor_reduce(
                out=q1r[:], in_=q1c[:], axis=mybir.AxisListType.X, op=AL.add
            )
            nc.vector.tensor_reduce(
                out=pack[:, 2:3], in_=s2c[:], axis=mybir.AxisListType.X, op=AL.add
            )
            nc.vector.tensor_reduce(
                out=pack[:, 3:4], in_=q2c[:], axis=mybir.AxisListType.X, op=AL.add
            )
            with tc.tile_pool(name="ppF", bufs=2, space="PSUM") as ppF:
                p8 = ppF.tile([8, 1], dt, tag="p8")
                nc.tensor.matmul(
                    out=p8[:], lhsT=fr(c_of[:]), rhs=fr(q1r[:]),
                    start=True, stop=True,
                )
                nc.scalar.copy(out=pack[0:8, 0:1], in_=p8[:])

            ar_in = dp.tile([128, 4], dt, tag="arin")
            ar_out = dp.tile([128, 4], dt, tag="arout")
            nc.gpsimd.dma_start(out=ar_in[:], in_=pack[:])
            nc.gpsimd.collective_compute(
                "AllReduce", AL.add,
                replica_groups=[list(range(NCORES))],
                ins=[ar_in.opt()], outs=[ar_out.opt()],
            )
            # keep the PE HAM window busy across the collective wait so the
            # sep conv doesn't start at the throttled clock
            with tc.tile_pool(name="ppJ", bufs=1, space="PSUM") as ppJ:
                pj = ppJ.tile([128, 512], dt, tag="pj")
                for _ in range(50):
                    nc.tensor.matmul(
                        out=pj[:], lhsT=c_sg[:, 0:128],
                        rhs=xw2[:, 0:512], start=True, stop=True,
                    )
            gst = st.tile([128, 4], dt, tag="gst")
            nc.gpsimd.dma_start(out=gst[:], in_=ar_out[:])
            if dbg:
                nc.sync.dma_start(out=dbg_out["d_gst"][:], in_=gst[:])

            if _PHASES < 4:
                return dram_in, out, dbg_out
            # ---- post-AR math: per-channel affine A, B ----
            tA = lambda tag: st.tile([128, 1], dt, tag=tag, name=tag)
            v1 = tA("v1"); r1 = tA("r1"); a2 = tA("a2"); al = tA("al")
            ta = tA("ta"); tb = tA("tb"); tcda = tA("tc")
            # rows 0..7: v1 = Q1/BNT - (S1/BNT)^2
            nc.vector.tensor_scalar_mul(out=ta[0:8, :], in0=gst[0:8, 0:1], scalar1=1.0 / BNT)
            nc.vector.tensor_scalar_mul(out=tb[0:8, :], in0=gst[0:8, 1:2], scalar1=1.0 / BNT)
            nc.vector.tensor_tensor(out=tcda[0:8, :], in0=tb[0:8, :], in1=tb[0:8, :], op=AL.mult)
            nc.vector.tensor_sub(out=v1[0:8, :], in0=ta[0:8, :], in1=tcda[0:8, :])
            nc.vector.tensor_scalar_add(out=v1[0:8, :], in0=v1[0:8, :], scalar1=EPS)
            nc.vector.reciprocal(out=r1[0:8, :], in_=v1[0:8, :])
            # a2 = g1^2 * r1 ; al = g1 * sqrt(r1)
            nc.vector.tensor_tensor(out=ta[0:8, :], in0=c_g1[:], in1=c_g1[:], op=AL.mult)
            nc.vector.tensor_tensor(out=a2[0:8, :], in0=ta[0:8, :], in1=r1[0:8, :], op=AL.mult)
            nc.scalar.activation(out=tb[0:8, :], in_=r1[0:8, :], func=AF.Sqrt)
            nc.vector.tensor_tensor(out=al[0:8, :], in0=c_g1[:], in1=tb[0:8, :], op=AL.mult)
            # broadcast to 128 channels via tiny matmuls
            a2c = tA("a2c"); alc = tA("alc")
            with tc.tile_pool(name="ppB", bufs=2, space="PSUM") as ppB:
                pb1 = ppB.tile([128, 1], dt, tag="pb1")
                nc.tensor.matmul(out=pb1[:], lhsT=fr(c_bc8[:]), rhs=fr(a2[0:8, :]), start=True, stop=True)
                nc.scalar.copy(out=a2c[:], in_=pb1[:])
                pb2 = ppB.tile([128, 1], dt, tag="pb1")
                nc.tensor.matmul(out=pb2[:], lhsT=fr(c_bc8[:]), rhs=fr(al[0:8, :]), start=True, stop=True)
                nc.scalar.copy(out=alc[:], in_=pb2[:])
            mg = tA("mg"); vg = tA("vg"); r2 = tA("r2"); Aff = tA("Aff"); Bff = tA("Bff")
            nc.vector.tensor_scalar_mul(out=mg[:], in0=gst[:, 2:3], scalar1=1.0 / BT)
            nc.vector.tensor_scalar_mul(out=vg[:], in0=gst[:, 3:4], scalar1=1.0 / BT)
            nc.vector.tensor_tensor(out=ta[:], in0=mg[:], in1=mg[:], op=AL.mult)
            nc.vector.tensor_sub(out=vg[:], in0=vg[:], in1=ta[:])
            nc.vector.tensor_tensor(out=vg[:], in0=vg[:], in1=a2c[:], op=AL.mult)
            nc.vector.tensor_scalar_add(out=vg[:], in0=vg[:], scalar1=EPS)
            nc.vector.reciprocal(out=r2[:], in_=vg[:])
            nc.scalar.activation(out=ta[:], in_=r2[:], func=AF.Sqrt)
            nc.vector.tensor_tensor(out=tb[:], in0=alc[:], in1=ta[:], op=AL.mult)
            nc.vector.tensor_tensor(out=Aff[:], in0=tb[:], in1=c_g2[:], op=AL.mult)
            nc.vector.tensor_tensor(out=ta[:], in0=Aff[:], in1=mg[:], op=AL.mult)
            nc.vector.tensor_sub(out=Bff[:], in0=c_b2[:], in1=ta[:])

            if _PHASES < 5:
                return dram_in, out, dbg_out
            # ---- affine + ELU + pool4 -> h_pad ----
            h_pad = pp.tile([128, 8 * 176], mdt, tag="hpad")
            nc.gpsimd.memset(h_pad[:].bitcast(F32), 0.0)
            with tc.tile_pool(name="elu", bufs=1) as ep:
                u_t = ep.tile([128, 5120], dt, tag="ut")
                e_t = ep.tile([128, 5120], dt, tag="et")
                pl1 = ep.tile([128, 2560], dt, tag="pl1")
                u_v = u_t[:].rearrange("p (c b t) -> p c b t", c=10, b=8)
                g_v = g_sb[:].rearrange("p (c b t) -> p c b t", c=10, b=8)
                e_v = e_t[:].rearrange("p (c b t) -> p c b t", c=10, b=8)
                p1_v = pl1[:].rearrange("p (c b t) -> p c b t", c=10, b=8)
                h_v = (
                    h_pad[:]
                    .rearrange("p (b w) -> p b w", b=8)[:, :, 7:167]
                    .rearrange("p b (c tl) -> p c b tl", c=10)
                )
                # per b-pair: affine+ELU+pool (ACT/DVE/GpSimd), handing each
                # pair to the sep conv (PE) as soon as it lands in h_pad
                for cb in range(4):
                    bs = slice(2 * cb, 2 * cb + 2)
                    nc.scalar.activation(
                        out=u_v[:, :, bs], in_=g_v[:, :, bs], func=AF.Identity,
                        scale=Aff[:, 0:1], bias=Bff[:, 0:1],
                    )
                    nc.vector.tensor_scalar_min(
                        out=g_v[:, :, bs], in0=u_v[:, :, bs], scalar1=0.0
                    )
                    nc.scalar.activation(
                        out=e_v[:, :, bs], in_=g_v[:, :, bs], func=AF.Exp
                    )
                    nc.vector.tensor_scalar_max(
                        out=u_v[:, :, bs], in0=u_v[:, :, bs], scalar1=0.0
                    )
                    nc.vector.scalar_tensor_tensor(
                        out=u_v[:, :, bs], in0=e_v[:, :, bs], scalar=-1.0,
                        in1=u_v[:, :, bs], op0=AL.add, op1=AL.add,
                    )
                    nc.gpsimd.tensor_add(
                        out=p1_v[:, :, bs], in0=u_v[:, :, bs, 0:64:2],
                        in1=u_v[:, :, bs, 1:64:2],
                    )
                    nc.vector.tensor_add(
                        out=h_v[:, :, bs], in0=p1_v[:, :, bs, 0:32:2],
                        in1=p1_v[:, :, bs, 1:32:2],
                    )
            if dbg:
                nc.sync.dma_start(out=dbg_out["d_h"][:].bitcast(h_pad[:].dtype), in_=h_pad[:])

            if _PHASES < 6:
                return dram_in, out, dbg_out
            # ---- sep conv + BN3 stats ----
            s_sb = pp.tile([128, 1280], dt, tag="ssb")
            q3c = st.tile([128, 4], dt, tag="q3c")
            pack2 = st.tile([128, 2], dt, tag="pack2")
            h_pv = h_pad[:].rearrange("p (b w) -> p b w", b=8)
            with tc.tile_pool(name="ppS", bufs=1, space="PSUM") as ppS:
                psb = [ppS.tile([128, 320], dt, tag=f"ps{cb}", name=f"ps{cb}") for cb in range(4)]
                # cb-outer so each b-pair's taps start as soon as its
                # h_pad slice is pooled
                for cb in range(4):
                    for tap in range(16):
                        rhs = h_pv[:, 2 * cb : 2 * cb + 2, tap : tap + 160]
                        nc.tensor.matmul(
                            out=psb[cb][:],
                            lhsT=fr(c_w2t[:, tap * 128 : (tap + 1) * 128]),
                            rhs=fr(rhs),
                            start=(tap == 0), stop=(tap == 15),
                        )
                s3tmp = st.tile([128, 4], dt, tag="s3tmp")
                for cb in range(4):
                    nc.scalar.activation(
                        out=s_sb[:, cb * 320 : (cb + 1) * 320], in_=psb[cb][:],
                        func=AF.Copy, accum_out=s3tmp[:, cb : cb + 1],
                    )
                    ssl = s_sb[:, cb * 320 : (cb + 1) * 320]
                    sv3 = scr.tile([128, 512], dt, tag="scrV2")
                    nc.vector.scalar_tensor_tensor(
                        out=sv3[:, 0:320], in0=ssl, scalar=0.0, in1=ssl,
                        op0=AL.add, op1=AL.mult,
                        accum_out=q3c[:, cb : cb + 1],
                    )
            nc.vector.tensor_reduce(
                out=pack2[:, 0:1], in_=s3tmp[:], axis=mybir.AxisListType.X, op=AL.add
            )
            nc.vector.tensor_reduce(
                out=pack2[:, 1:2], in_=q3c[:], axis=mybir.AxisListType.X, op=AL.add
            )
            if dbg:
                nc.sync.dma_start(out=dbg_out["d_s"][:], in_=s_sb[:])

            ar2_in = dp.tile([128, 2], dt, tag="ar2in")
            ar2_out = dp.tile([128, 2], dt, tag="ar2out")
            nc.gpsimd.dma_start(out=ar2_in[:], in_=pack2[:])
            nc.gpsimd.collective_compute(
                "AllReduce", AL.add,
                replica_groups=[list(range(NCORES))],
                ins=[ar2_in.opt()], outs=[ar2_out.opt()],
            )
            gst2 = st.tile([128, 2], dt, tag="gst2")
            nc.gpsimd.dma_start(out=gst2[:], in_=ar2_out[:])

            m3 = tA("m3"); v3 = tA("v3"); A3 = tA("A3"); B3 = tA("B3")
            nc.vector.tensor_scalar_mul(out=m3[:], in0=gst2[:, 0:1], scalar1=1.0 / BT2)
            nc.vector.tensor_scalar_mul(out=v3[:], in0=gst2[:, 1:2], scalar1=1.0 / BT2)
            nc.vector.tensor_tensor(out=ta[:], in0=m3[:], in1=m3[:], op=AL.mult)
            nc.vector.tensor_sub(out=v3[:], in0=v3[:], in1=ta[:])
            nc.vector.tensor_scalar_add(out=v3[:], in0=v3[:], scalar1=EPS)
            nc.vector.reciprocal(out=ta[:], in_=v3[:])
            nc.scalar.activation(out=tb[:], in_=ta[:], func=AF.Sqrt)
            nc.vector.tensor_tensor(out=A3[:], in0=tb[:], in1=c_g3[:], op=AL.mult)
            nc.vector.tensor_tensor(out=ta[:], in0=A3[:], in1=m3[:], op=AL.mult)
            nc.vector.tensor_sub(out=B3[:], in0=c_b3[:], in1=ta[:])

            if _PHASES < 7:
                return dram_in, out, dbg_out
            # ---- BN3 affine + ELU + pool8 -> h3 ----
            h3 = pp.tile([128, 160], dt, tag="h3")
            with tc.tile_pool(name="elu3", bufs=1) as e3p:
                v3t = e3p.tile([128, 1280], dt, tag="v3t")
                e3t = e3p.tile([128, 1280], dt, tag="e3t")
                nc.scalar.activation(
                    out=v3t[:], in_=s_sb[:], func=AF.Identity,
                    scale=A3[:, 0:1], bias=B3[:, 0:1],
                )
                nc.vector.tensor_scalar_min(out=s_sb[:], in0=v3t[:], scalar1=0.0)
                nc.scalar.activation(out=e3t[:], in_=s_sb[:], func=AF.Exp)
                nc.vector.tensor_scalar_max(out=v3t[:], in0=v3t[:], scalar1=0.0)
                nc.vector.scalar_tensor_tensor(
                    out=v3t[:], in0=e3t[:], scalar=-1.0, in1=v3t[:],
                    op0=AL.add, op1=AL.add,
                )
                pq1 = e3p.tile([128, 640], dt, tag="pq1")
                pq2 = e3p.tile([128, 320], dt, tag="pq2")
                nc.gpsimd.tensor_add(
                    out=pq1[:], in0=v3t[:, 0:1280:2], in1=v3t[:, 1:1280:2]
                )
                nc.vector.tensor_add(
                    out=pq2[:], in0=pq1[:, 0:640:2], in1=pq1[:, 1:640:2]
                )
                nc.gpsimd.tensor_add(
                    out=h3[:], in0=pq2[:, 0:320:2], in1=pq2[:, 1:320:2]
                )
            if dbg:
                nc.sync.dma_start(out=dbg_out["d_h3"][:], in_=h3[:])

            if _PHASES < 8:
                return dram_in, out, dbg_out
            # ---- FC ----
            h3_v = h3[:].rearrange("p (b t) -> p b t", b=8)
            o_sb = st.tile([4, 8], dt, tag="osb")
            with tc.tile_pool(name="ppO", bufs=1, space="PSUM") as ppO:
                po = ppO.tile([4, 8], dt, tag="po")
                for t2 in range(20):
                    nc.tensor.matmul(
                        out=po[:],
                        lhsT=fr(c_fcw[:, t2 * 4 : (t2 + 1) * 4]),
                        rhs=fr(h3_v[:, :, t2]),
                        start=(t2 == 0), stop=(t2 == 19),
                    )
                nc.scalar.copy(out=o_sb[:], in_=po[:])
            nc.vector.tensor_scalar_add(
                out=o_sb[:], in0=o_sb[:], scalar1=c_fcb[0:4, 0:1]
            )
            nc.sync.dma_start(out=out.rearrange("b j -> j b"), in_=o_sb[:])

    return dram_in, out, dbg_out


def build_and_run(inputs, dbg=False, trace=False):
    _patch_tile_tail()
    X = np.asarray(inputs["X"], np.float32)
    consts = _host_tensors(
        inputs["L"], inputs["conv1_w"], inputs["cheb_w"],
        inputs["sep_w"], inputs["fc_w"],
    )
    common = dict(consts)
    common["g1"] = np.ascontiguousarray(np.asarray(inputs["bn1_g"], np.float32).reshape(8, 1))
    common["g2"] = np.ascontiguousarray(np.asarray(inputs["bn2_g"], np.float32).reshape(128, 1))
    common["b2"] = np.ascontiguousarray(np.asarray(inputs["bn2_b"], np.float32).reshape(128, 1))
    common["g3"] = np.ascontiguousarray(np.asarray(inputs["bn3_g"], np.float32).reshape(128, 1))
    common["b3"] = np.ascontiguousarray(np.asarray(inputs["bn3_b"], np.float32).reshape(128, 1))
    common["fcb"] = np.ascontiguousarray(np.asarray(inputs["fc_b"], np.float32).reshape(4, 1))

    nc = bass.Bass("TRN2", target_bir_lowering=False)
    _emit(nc, dbg=dbg)
    _split_waits(nc)

    in_maps = []
    for core in range(NCORES):
        m = dict(common)
        m["xs"] = np.ascontiguousarray(
            X[core * BL : (core + 1) * BL].reshape(4, 128, 640)
        )
        in_maps.append(m)
    res = run_bass_kernel_spmd(nc, in_maps, list(range(NCORES)), trace=trace)
    out = np.concatenate([res.results[i]["out"] for i in range(NCORES)], axis=0)
    return out.astype(np.float32), res


def kernel(**inputs):
    out, _ = build_and_run(inputs)
    return out
